# revision 8
# baseline (speedup 1.0000x reference)
"""MoELoRA forward kernel for 8x Trainium2 NeuronCores (Bass/Tile).

Math (see reference):
  route   = softmax(x @ W_route^T)                      [N, E]
  h       = x @ A[e,g,r,:]^T                            [N, E, G, R]
  wh      = h * route[..., None, None]
  compact = einsum(wh, Bw[e,g,o,r]) * SCALING           [N, G, OD]
  out     = zeros([N, OUT]); out[:, lora_ind] = compact.reshape(N, G*OD)

Device strategy (data-parallel over tokens, weights replicated):
  - Host pre-transposes each x shard to fp8-e3m4 xT [D, TPC]: the kernel is
    DMA-bandwidth-bound and x is the dominant input, so e3m4 (4 mantissa
    bits; empirically 1.35e-2 max rel err end-to-end vs the 2e-2 budget)
    halves the x read traffic. The contraction dim d lands on SBUF
    partitions with contiguous 512B DMA lines.
  - A is reordered to feature-major f = (g, e, r) and concatenated with
    W_route^T into one fp16 [128, KD, FE] rhs, pre-arranged on the host so
    each partition's DMA line is fully contiguous (2176B) for full-rate DMA.
    One accumulated matmul chain per 128-token tile produces h (cols
    0..127) and the routing logits (cols 128..135); fp8 lhsT x fp16 rhs is
    a legal mixed-precision matmul.
  - Softmax: exp (no max-subtract; logits are O(1)) with the row-sum fused
    into the ACT instruction via accum_out, one reciprocal, then
    probs = expv * rsum so the final PSUM->SBUF copies are scale-free and
    can run on any engine. SCALING=2 is folded into B on the host.
  - wh = h * probs uses a step-0 broadcast access pattern; wh is
    PE-transposed once per tile and the per-group up-projection runs as
    TWO K=128 matmuls of free-size 1024 against a block-diagonal fp16
    [128, 2048] B (fewer PE-SEQ instructions; PE.SEQ is near-critical).
  - The two [128,1024] fp32->fp16 PSUM drains go to Act and Pool (DVE
    keeps the softmax/wh chain), keeping every engine under the ~1.92us
    per-tile DMA cadence.
  - compact is staged fp16 in SBUF and DMAed out fp16 (halves the dominant
    write); the host upcasts and performs the lora_ind zero-pad scatter
    during unsharding.
"""

import sys
from concurrent.futures import ThreadPoolExecutor
from contextlib import ExitStack

for _p in ("/opt/trn_rl_repo", "/root/.axon_site/_ro/trn_rl_repo"):
    if _p not in sys.path:
        sys.path.insert(0, _p)

import ml_dtypes
import numpy as np

import concourse.bass as bass  # noqa: F401
import concourse.mybir as mybir
import concourse.tile as tile
from concourse import bacc
from concourse.bass_utils import run_bass_kernel_spmd
from concourse.masks import make_identity

# Problem dims (hardcoded per spec nn_MoELoRA_28089086116115)
B, S, D = 4, 4096, 1024
OUT = 3072
R, E, G = 8, 8, 2
OD = OUT // 3                    # 1024
F = G * E * R                    # 128 lora features, f = g*64 + e*8 + r
FE = F + E                       # 136: features + routing logits
SCALING = 16.0 / 8.0
NCORES = 8
NTOK = B * S                     # 16384
TPC = NTOK // NCORES             # 2048 tokens per core
TBLK = 512                       # tokens per x DMA block
NBLK = TPC // TBLK
KD = D // 128                    # 8 contraction chunks

F8 = ml_dtypes.float8_e3m4

# Hooks for test.py (not used by the grader, which calls kernel() only).
_RUN_KWARGS: dict = {}
_LAST: dict = {}

_nc_cache = None


def _build():
    f32 = mybir.dt.float32
    f16 = mybir.dt.float16
    f8 = mybir.dt.float8e3
    Exp = mybir.ActivationFunctionType.Exp
    Copy = mybir.ActivationFunctionType.Copy
    mult = mybir.AluOpType.mult

    nc = bacc.Bacc("TRN2", target_bir_lowering=False, debug=False,
                   num_devices=NCORES)
    xT = nc.dram_tensor("xT", [D, TPC], f8, kind="ExternalInput")
    awt = nc.dram_tensor("AWT", [128, KD, FE], f16, kind="ExternalInput")
    btbd = nc.dram_tensor("BT", [G, E * R, OD], f16, kind="ExternalInput")
    out = nc.dram_tensor("out", [TPC, G * OD], f16, kind="ExternalOutput")

    with tile.TileContext(nc) as tc, ExitStack() as ctx:
        wp = ctx.enter_context(tc.tile_pool(name="wp", bufs=1))
        awt_sb = wp.tile([128, KD, FE], f16)

        bt_sb = wp.tile([128, G * OD], f16)
        nc.gpsimd.memset(bt_sb[:], 0.0)
        ident = wp.tile([128, 128], f16)
        make_identity(nc, ident)

        # all x blocks live in SBUF at once (4 x 4KB/partition, fp8)
        xp = ctx.enter_context(tc.tile_pool(name="xp", bufs=NBLK))
        sp = ctx.enter_context(tc.tile_pool(name="sp", bufs=8))
        outp = ctx.enter_context(tc.tile_pool(name="outp", bufs=4))
        ph = ctx.enter_context(tc.tile_pool(name="ph", bufs=2, space="PSUM"))
        pt = ctx.enter_context(tc.tile_pool(name="pt", bufs=2, space="PSUM"))
        pc = ctx.enter_context(tc.tile_pool(name="pc", bufs=2, space="PSUM"))

        # weights first (compute needs awt + x block 0), then all x reads
        # up-front so no read ever queues behind a compute-gated write.
        nc.sync.dma_start(awt_sb[:], awt[:])
        x_sbs = []
        for blk in range(NBLK):
            x_sb = xp.tile([128, KD, TBLK], f8, name=f"x{blk}")
            xr = xT[:, blk * TBLK:(blk + 1) * TBLK].rearrange(
                "(k p) t -> p k t", p=128)
            nc.sync.dma_start(x_sb[:], xr)
            x_sbs.append(x_sb)
            if blk == 0:
                # BT is block-diagonal: zero the tile (idle Pool engine) and
                # DMA only the two nonzero 128KB blocks.
                nc.sync.dma_start(bt_sb[0:64, 0:1024], btbd[0])
                nc.sync.dma_start(bt_sb[64:128, 1024:2048], btbd[1])

        # Two-stage software pipeline over the 16 128-token subtiles: iter N
        # runs the h-matmuls + softmax/wh for subtile N while transposing,
        # up-projecting and draining subtile N-1. This keeps every engine's
        # in-order queue free of head-of-line stalls (the drains land after
        # the same-iteration up-proj matmuls, the transpose input is a full
        # iteration old) so the steady-state cadence is DMA-paced.
        NSUB = TPC // 128
        wh_t = [None] * NSUB       # wh tiles (SBUF fp16), stage S2 output
        o_sbs = [None] * (NSUB // 2)

        for N in range(NSUB + 1):
            if N < NSUB:
                x_sb = x_sbs[N // 4]
                t0 = (N % 4) * 128
                # S1: h (cols 0..127) + routing logits (cols 128..135)
                hE = ph.tile([128, FE], f32)
                for k in range(KD):
                    nc.tensor.matmul(
                        hE[:],
                        lhsT=x_sb[:, k, t0:t0 + 128],
                        rhs=awt_sb[:, k, :],
                        start=(k == 0),
                        stop=(k == KD - 1),
                    )
                # S2: softmax probs = exp(logits) / sum, then wh = h * probs
                expv = sp.tile([128, E], f32)
                ssum = sp.tile([128, 1], f32)
                nc.scalar.activation(expv[:], hE[:, F:FE], Exp,
                                     accum_out=ssum[:, 0:1])
                rsum = sp.tile([128, 1], f32)
                nc.vector.reciprocal(rsum[:], ssum[:])
                probs = sp.tile([128, E], f32)
                nc.gpsimd.tensor_scalar_mul(probs[:], expv[:], rsum[:, 0:1])
                wh = sp.tile([128, F], f16)
                nc.vector.tensor_tensor(
                    out=wh.rearrange("p (g e r) -> p g e r", g=G, e=E),
                    in0=hE[:, 0:F].rearrange("p (g e r) -> p g e r", g=G, e=E),
                    in1=probs[:, None, :, None].to_broadcast([128, G, E, R]),
                    op=mult,
                )
                wh_t[N] = wh

            M = N - 1
            if 0 <= M < NSUB:
                half = M % 2
                if half == 0:
                    o_sbs[M // 2] = outp.tile([128, 2, G * OD], f16,
                                              name=f"o{M // 2}", tag="o")
                o_sb = o_sbs[M // 2]
                # S3: transpose so the (g,e,r) contraction lands on partitions
                whT_ps = pt.tile([128, 128], f16)
                nc.tensor.transpose(whT_ps[:], wh_t[M][:], ident[:])
                whT = sp.tile([128, 128], f16)
                nc.vector.tensor_copy(whT[:], whT_ps[:])
                # S4: compact[t, (g,o)] via block-diagonal 2*B^T (K=128),
                # one PSUM bank per 512-col matmul
                cps_l = []
                for j in range(4):
                    cps = pc.tile([128, 512], f32, name=f"cps{j}", tag="cps")
                    nc.tensor.matmul(
                        cps[:],
                        lhsT=whT[:],
                        rhs=bt_sb[:, j * 512:(j + 1) * 512],
                        start=True,
                        stop=True,
                    )
                    cps_l.append(cps)
                # S5: plain fp32->fp16 PSUM drains (gpsimd cannot read PSUM
                # on this target): Act gets 2.5 per subtile, DVE 1.5
                for j in range(4):
                    dst = o_sb[:, half, j * 512:(j + 1) * 512]
                    on_act = (j in (0, 2)) or (j == 3 and M % 2 == 0)
                    if on_act:
                        nc.scalar.activation(dst, cps_l[j][:], Copy)
                    else:
                        nc.vector.tensor_copy(dst, cps_l[j][:])
                # S6: output writes. Edge pairs go out per-subtile (the first
                # writes start a subtile earlier; the final write is half as
                # long); steady-state pairs share one 1 MiB write.
                pair = M // 2
                edge = pair <= 2 or pair >= NSUB // 2 - 2
                r0 = pair * 256
                if edge:
                    nc.sync.dma_start(
                        out[r0 + half * 128:r0 + (half + 1) * 128, :],
                        o_sb[:, half, :])
                elif half == 1:
                    nc.sync.dma_start(
                        out[r0:r0 + 256, :].rearrange(
                            "(s p) o -> p s o", p=128),
                        o_sb[:])

    nc.compile()
    return nc


def _shard_xT(x, c):
    return np.ascontiguousarray(x[c * TPC:(c + 1) * TPC].T).astype(F8)


_runner = None


def _get_runner(nc):
    """Build the sharded PJRT callable once; reuse across kernel() calls.

    Mirrors bass2jax.run_bass_via_pjrt's multi-core branch, but caches the
    jitted function so repeat calls skip retrace/recompile. Falls back to
    the stock path (handled by caller) on any failure.
    """
    global _runner
    if _runner is not None:
        return _runner
    import jax
    from jax.experimental.shard_map import shard_map
    from jax.sharding import Mesh, PartitionSpec

    from concourse import bass2jax, mybir as _mb

    bass2jax.install_neuronx_cc_hook()
    partition_name = (nc.partition_id_tensor.name
                      if nc.partition_id_tensor else None)
    in_names, out_names, out_avals = [], [], []
    for alloc in nc.m.functions[0].allocations:
        if not isinstance(alloc, _mb.MemoryLocationSet):
            continue
        name = alloc.memorylocations[0].name
        if alloc.kind == "ExternalInput":
            if name != partition_name:
                in_names.append(name)
        elif alloc.kind == "ExternalOutput":
            out_names.append(name)
            out_avals.append(jax.core.ShapedArray(
                tuple(alloc.tensor_shape), _mb.dt.np(alloc.dtype)))
    n_params = len(in_names)
    n_outs = len(out_avals)
    all_in_names = list(in_names) + list(out_names)
    if partition_name is not None:
        all_in_names.append(partition_name)

    def _body(*args):
        operands = list(args)
        if partition_name is not None:
            operands.append(bass2jax.partition_id_tensor())
        outs = bass2jax._bass_exec_p.bind(
            *operands,
            out_avals=tuple(out_avals),
            in_names=tuple(all_in_names),
            out_names=tuple(out_names),
            lowering_input_output_aliases=(),
            sim_require_finite=True,
            sim_require_nnan=True,
            nc=nc,
        )
        return tuple(outs)

    devices = jax.devices()[:NCORES]
    mesh = Mesh(np.asarray(devices), ("core",))
    specs = (PartitionSpec("core"),) * (n_params + n_outs)
    sharded = jax.jit(
        shard_map(_body, mesh=mesh, in_specs=specs,
                  out_specs=(PartitionSpec("core"),) * n_outs,
                  check_rep=False),
        donate_argnums=tuple(range(n_params, n_params + n_outs)),
        keep_unused=True,
    )
    _runner = (sharded, in_names, out_names, out_avals)
    return _runner


def _run_cached(nc, in_maps):
    sharded, in_names, out_names, out_avals = _get_runner(nc)
    concat_in = [
        np.concatenate([np.asarray(m[name]) for m in in_maps], axis=0)
        for name in in_names
    ]
    concat_zeros = [
        np.zeros((NCORES * a.shape[0], *a.shape[1:]), a.dtype)
        for a in out_avals
    ]
    out_arrs = sharded(*concat_in, *concat_zeros)
    return [
        {name: np.asarray(out_arrs[i]).reshape(NCORES, *out_avals[i].shape)[c]
         for i, name in enumerate(out_names)}
        for c in range(NCORES)
    ]


def kernel(x, W_route, A, Bw, lora_ind):
    global _nc_cache
    x = np.asarray(x, dtype=np.float32).reshape(NTOK, D)
    W_route = np.asarray(W_route, dtype=np.float32)
    A = np.asarray(A, dtype=np.float32)
    Bw = np.asarray(Bw, dtype=np.float32)
    lora_ind = np.asarray(lora_ind).astype(np.int64)

    # [D, 136] fp16: cols 0..127 are A rows in (g, e, r) order, 128.. W_route;
    # re-packed to [128, KD, FE] so each partition's DMA line is contiguous.
    A_all = A.transpose(1, 0, 2, 3).reshape(F, D)
    AWT = np.concatenate([A_all.T, W_route.T], axis=1).astype(np.float16)
    AWT_dev = np.ascontiguousarray(
        AWT.reshape(KD, 128, FE).transpose(1, 0, 2))
    # block-diagonal B^T with SCALING folded in: rows (g,e,r), cols (g,o)
    BTbd = (Bw.transpose(1, 0, 3, 2).reshape(G, E * R, OD)
            * SCALING).astype(np.float16)

    if _nc_cache is None:
        _nc_cache = _build()
    nc = _nc_cache

    with ThreadPoolExecutor(NCORES) as ex:
        xTs = list(ex.map(lambda c: _shard_xT(x, c), range(NCORES)))
    in_maps = [{"xT": xTs[c], "AWT": AWT_dev, "BT": BTbd}
               for c in range(NCORES)]

    try:
        results = _run_cached(nc, in_maps)
    except Exception:  # noqa: BLE001  (fall back to the stock SPMD path)
        global _runner
        _runner = None
        res = run_bass_kernel_spmd(nc, in_maps, core_ids=list(range(NCORES)),
                                   **_RUN_KWARGS)
        results = res.results
    _LAST["results"] = results

    compact = np.concatenate(
        [results[c]["out"] for c in range(NCORES)], axis=0)
    outp = np.zeros((NTOK, OUT), dtype=np.float32)
    outp[:, lora_ind] = compact.astype(np.float32)
    return outp.reshape(B, S, OUT)


# revision 11
# speedup vs baseline: 1.0579x; 1.0579x over previous
"""MoELoRA forward kernel for 8x Trainium2 NeuronCores (Bass/Tile).

Math (see reference):
  route   = softmax(x @ W_route^T)                      [N, E]
  h       = x @ A[e,g,r,:]^T                            [N, E, G, R]
  wh      = h * route[..., None, None]
  compact = einsum(wh, Bw[e,g,o,r]) * SCALING           [N, G, OD]
  out     = zeros([N, OUT]); out[:, lora_ind] = compact.reshape(N, G*OD)

Device strategy (data-parallel over tokens, weights replicated):
  - Host pre-transposes each x shard to fp8-e3m4 xT [D, TPC]: the kernel is
    DMA-bandwidth-bound and x is the dominant input, so e3m4 (4 mantissa
    bits; empirically 1.35e-2 max rel err end-to-end vs the 2e-2 budget)
    halves the x read traffic. The contraction dim d lands on SBUF
    partitions with contiguous 512B DMA lines.
  - A is reordered to feature-major f = (g, e, r) and concatenated with
    W_route^T into one fp16 [128, KD, FE] rhs, pre-arranged on the host so
    each partition's DMA line is fully contiguous (2176B) for full-rate DMA.
    One accumulated matmul chain per 128-token tile produces h (cols
    0..127) and the routing logits (cols 128..135); fp8 lhsT x fp16 rhs is
    a legal mixed-precision matmul.
  - Softmax: exp (no max-subtract; logits are O(1)) with the row-sum fused
    into the ACT instruction via accum_out, one reciprocal, then
    probs = expv * rsum so the final PSUM->SBUF copies are scale-free and
    can run on any engine. SCALING=2 is folded into B on the host.
  - wh = h * probs uses a step-0 broadcast access pattern; wh is
    PE-transposed once per tile and the per-group up-projection runs as
    TWO K=128 matmuls of free-size 1024 against a block-diagonal fp16
    [128, 2048] B (fewer PE-SEQ instructions; PE.SEQ is near-critical).
  - The two [128,1024] fp32->fp16 PSUM drains go to Act and Pool (DVE
    keeps the softmax/wh chain), keeping every engine under the ~1.92us
    per-tile DMA cadence.
  - compact is staged fp16 in SBUF and DMAed out fp16 (halves the dominant
    write); the host upcasts and performs the lora_ind zero-pad scatter
    during unsharding.
"""

import sys
from concurrent.futures import ThreadPoolExecutor
from contextlib import ExitStack

for _p in ("/opt/trn_rl_repo", "/root/.axon_site/_ro/trn_rl_repo"):
    if _p not in sys.path:
        sys.path.insert(0, _p)

import ml_dtypes
import numpy as np

import concourse.bass as bass  # noqa: F401
import concourse.mybir as mybir
import concourse.tile as tile
from concourse import bacc
from concourse.bass_utils import run_bass_kernel_spmd
from concourse.masks import make_identity

# Problem dims (hardcoded per spec nn_MoELoRA_28089086116115)
B, S, D = 4, 4096, 1024
OUT = 3072
R, E, G = 8, 8, 2
OD = OUT // 3                    # 1024
F = G * E * R                    # 128 lora features, f = g*64 + e*8 + r
FE = F + E                       # 136: features + routing logits
SCALING = 16.0 / 8.0
NCORES = 8
NTOK = B * S                     # 16384
TPC = NTOK // NCORES             # 2048 tokens per core
TBLK = 512                       # tokens per x DMA block
NBLK = TPC // TBLK
KD = D // 128                    # 8 contraction chunks

F8 = ml_dtypes.float8_e3m4

# Hooks for test.py (not used by the grader, which calls kernel() only).
_RUN_KWARGS: dict = {}
_LAST: dict = {}

_nc_cache = None


def _build():
    f32 = mybir.dt.float32
    f16 = mybir.dt.float16
    f8 = mybir.dt.float8e3
    Exp = mybir.ActivationFunctionType.Exp
    Copy = mybir.ActivationFunctionType.Copy
    mult = mybir.AluOpType.mult

    nc = bacc.Bacc("TRN2", target_bir_lowering=False, debug=False,
                   num_devices=NCORES)
    xT = nc.dram_tensor("xT", [D, TPC], f8, kind="ExternalInput")
    awt = nc.dram_tensor("AWT", [128, KD, FE], f16, kind="ExternalInput")
    btbd = nc.dram_tensor("BT", [G, E * R, OD], f16, kind="ExternalInput")
    out = nc.dram_tensor("out", [TPC, G * OD], f16, kind="ExternalOutput")

    with tile.TileContext(nc) as tc, ExitStack() as ctx:
        wp = ctx.enter_context(tc.tile_pool(name="wp", bufs=1))
        awt_sb = wp.tile([128, KD, FE], f16)

        bt_sb = wp.tile([128, G * OD], f16)
        nc.gpsimd.memset(bt_sb[:], 0.0)
        ident = wp.tile([128, 128], f16)
        make_identity(nc, ident)

        # all x blocks live in SBUF at once (4 x 4KB/partition, fp8)
        xp = ctx.enter_context(tc.tile_pool(name="xp", bufs=NBLK))
        sp = ctx.enter_context(tc.tile_pool(name="sp", bufs=8))
        outp = ctx.enter_context(tc.tile_pool(name="outp", bufs=5))
        ph = ctx.enter_context(tc.tile_pool(name="ph", bufs=2, space="PSUM"))
        pt = ctx.enter_context(tc.tile_pool(name="pt", bufs=2, space="PSUM"))
        pc = ctx.enter_context(tc.tile_pool(name="pc", bufs=2, space="PSUM"))

        # weights first (compute needs awt + x block 0), then all x reads
        # up-front so no read ever queues behind a compute-gated write.
        nc.sync.dma_start(awt_sb[:], awt[:])
        x_sbs = []
        for blk in range(NBLK):
            x_sb = xp.tile([128, KD, TBLK], f8, name=f"x{blk}")
            xr = xT[:, blk * TBLK:(blk + 1) * TBLK].rearrange(
                "(k p) t -> p k t", p=128)
            if blk == 0:
                # split block 0 along k so the first h-matmuls start half a
                # block earlier (each k-line stays a full-rate 512B descriptor)
                nc.sync.dma_start(x_sb[:, 0:KD // 2, :], xr[:, 0:KD // 2, :])
                nc.sync.dma_start(x_sb[:, KD // 2:, :], xr[:, KD // 2:, :])
            else:
                nc.sync.dma_start(x_sb[:], xr)
            x_sbs.append(x_sb)
            if blk == 0:
                # BT is block-diagonal: zero the tile (idle Pool engine) and
                # DMA only the two nonzero 128KB blocks.
                nc.sync.dma_start(bt_sb[0:64, 0:1024], btbd[0])
                nc.sync.dma_start(bt_sb[64:128, 1024:2048], btbd[1])

        # Two-stage software pipeline over the 16 128-token subtiles: iter N
        # runs the h-matmuls + softmax/wh for subtile N while transposing,
        # up-projecting and draining subtile N-1. This keeps every engine's
        # in-order queue free of head-of-line stalls (the drains land after
        # the same-iteration up-proj matmuls, the transpose input is a full
        # iteration old) so the steady-state cadence is DMA-paced.
        NSUB = TPC // 128
        wh_t = [None] * NSUB       # wh tiles (SBUF fp16), stage S2 output
        o_sbs = [None] * (NSUB // 2)

        for N in range(NSUB + 1):
            if N < NSUB:
                x_sb = x_sbs[N // 4]
                t0 = (N % 4) * 128
                # S1: h (cols 0..127) + routing logits (cols 128..135)
                hE = ph.tile([128, FE], f32)
                for k in range(KD):
                    nc.tensor.matmul(
                        hE[:],
                        lhsT=x_sb[:, k, t0:t0 + 128],
                        rhs=awt_sb[:, k, :],
                        start=(k == 0),
                        stop=(k == KD - 1),
                    )
                # S2: softmax probs = exp(logits) / sum, then wh = h * probs
                expv = sp.tile([128, E], f32)
                ssum = sp.tile([128, 1], f32)
                # plain exp (no accum_out: the accumulator read costs Act an
                # extra 187ns and Act is the tightest engine); sum on DVE
                nc.scalar.activation(expv[:], hE[:, F:FE], Exp)
                nc.vector.reduce_sum(ssum[:], expv[:],
                                     axis=mybir.AxisListType.X)
                rsum = sp.tile([128, 1], f32)
                nc.vector.reciprocal(rsum[:], ssum[:])
                probs = sp.tile([128, E], f32)
                nc.gpsimd.tensor_scalar_mul(probs[:], expv[:], rsum[:, 0:1])
                wh = sp.tile([128, F], f16)
                nc.vector.tensor_tensor(
                    out=wh.rearrange("p (g e r) -> p g e r", g=G, e=E),
                    in0=hE[:, 0:F].rearrange("p (g e r) -> p g e r", g=G, e=E),
                    in1=probs[:, None, :, None].to_broadcast([128, G, E, R]),
                    op=mult,
                )
                wh_t[N] = wh

            M = N - 1
            if 0 <= M < NSUB:
                half = M % 2
                if half == 0:
                    o_sbs[M // 2] = outp.tile([128, 2, G * OD], f16,
                                              name=f"o{M // 2}", tag="o")
                o_sb = o_sbs[M // 2]
                # S3: transpose so the (g,e,r) contraction lands on partitions
                whT_ps = pt.tile([128, 128], f16)
                nc.tensor.transpose(whT_ps[:], wh_t[M][:], ident[:])
                whT = sp.tile([128, 128], f16)
                nc.vector.tensor_copy(whT[:], whT_ps[:])
                # S4: compact[t, (g,o)] via block-diagonal 2*B^T (K=128),
                # one PSUM bank per 512-col matmul
                cps_l = []
                for j in range(4):
                    cps = pc.tile([128, 512], f32, name=f"cps{j}", tag="cps")
                    nc.tensor.matmul(
                        cps[:],
                        lhsT=whT[:],
                        rhs=bt_sb[:, j * 512:(j + 1) * 512],
                        start=True,
                        stop=True,
                    )
                    cps_l.append(cps)
                # S5: plain fp32->fp16 PSUM drains (gpsimd cannot read PSUM
                # on this target): Act gets 2.5 per subtile, DVE 1.5
                for j in range(4):
                    dst = o_sb[:, half, j * 512:(j + 1) * 512]
                    on_act = (j in (0, 2)) or (j == 3 and M % 2 == 0)
                    if on_act:
                        nc.scalar.activation(dst, cps_l[j][:], Copy)
                    else:
                        nc.vector.tensor_copy(dst, cps_l[j][:])
                # S6: output writes. Edge pairs go out per-subtile (the first
                # writes start a subtile earlier; the final write is half as
                # long); steady-state pairs share one 1 MiB write.
                pair = M // 2
                edge = pair <= 2 or pair >= NSUB // 2 - 2
                r0 = pair * 256
                if edge:
                    nc.sync.dma_start(
                        out[r0 + half * 128:r0 + (half + 1) * 128, :],
                        o_sb[:, half, :])
                elif half == 1:
                    nc.sync.dma_start(
                        out[r0:r0 + 256, :].rearrange(
                            "(s p) o -> p s o", p=128),
                        o_sb[:])

    nc.compile()
    return nc


def _shard_xT(x, c):
    return np.ascontiguousarray(x[c * TPC:(c + 1) * TPC].T).astype(F8)


_runner = None


def _get_runner(nc):
    """Build the sharded PJRT callable once; reuse across kernel() calls.

    Mirrors bass2jax.run_bass_via_pjrt's multi-core branch, but caches the
    jitted function so repeat calls skip retrace/recompile. Falls back to
    the stock path (handled by caller) on any failure.
    """
    global _runner
    if _runner is not None:
        return _runner
    import jax
    from jax.experimental.shard_map import shard_map
    from jax.sharding import Mesh, PartitionSpec

    from concourse import bass2jax, mybir as _mb

    bass2jax.install_neuronx_cc_hook()
    partition_name = (nc.partition_id_tensor.name
                      if nc.partition_id_tensor else None)
    in_names, out_names, out_avals = [], [], []
    for alloc in nc.m.functions[0].allocations:
        if not isinstance(alloc, _mb.MemoryLocationSet):
            continue
        name = alloc.memorylocations[0].name
        if alloc.kind == "ExternalInput":
            if name != partition_name:
                in_names.append(name)
        elif alloc.kind == "ExternalOutput":
            out_names.append(name)
            out_avals.append(jax.core.ShapedArray(
                tuple(alloc.tensor_shape), _mb.dt.np(alloc.dtype)))
    n_params = len(in_names)
    n_outs = len(out_avals)
    all_in_names = list(in_names) + list(out_names)
    if partition_name is not None:
        all_in_names.append(partition_name)

    def _body(*args):
        operands = list(args)
        if partition_name is not None:
            operands.append(bass2jax.partition_id_tensor())
        outs = bass2jax._bass_exec_p.bind(
            *operands,
            out_avals=tuple(out_avals),
            in_names=tuple(all_in_names),
            out_names=tuple(out_names),
            lowering_input_output_aliases=(),
            sim_require_finite=True,
            sim_require_nnan=True,
            nc=nc,
        )
        return tuple(outs)

    devices = jax.devices()[:NCORES]
    mesh = Mesh(np.asarray(devices), ("core",))
    specs = (PartitionSpec("core"),) * (n_params + n_outs)
    sharded = jax.jit(
        shard_map(_body, mesh=mesh, in_specs=specs,
                  out_specs=(PartitionSpec("core"),) * n_outs,
                  check_rep=False),
        donate_argnums=tuple(range(n_params, n_params + n_outs)),
        keep_unused=True,
    )
    _runner = (sharded, in_names, out_names, out_avals)
    return _runner


def _run_cached(nc, in_maps):
    sharded, in_names, out_names, out_avals = _get_runner(nc)
    concat_in = [
        np.concatenate([np.asarray(m[name]) for m in in_maps], axis=0)
        for name in in_names
    ]
    concat_zeros = [
        np.zeros((NCORES * a.shape[0], *a.shape[1:]), a.dtype)
        for a in out_avals
    ]
    out_arrs = sharded(*concat_in, *concat_zeros)
    return [
        {name: np.asarray(out_arrs[i]).reshape(NCORES, *out_avals[i].shape)[c]
         for i, name in enumerate(out_names)}
        for c in range(NCORES)
    ]


def kernel(x, W_route, A, Bw, lora_ind):
    global _nc_cache
    x = np.asarray(x, dtype=np.float32).reshape(NTOK, D)
    W_route = np.asarray(W_route, dtype=np.float32)
    A = np.asarray(A, dtype=np.float32)
    Bw = np.asarray(Bw, dtype=np.float32)
    lora_ind = np.asarray(lora_ind).astype(np.int64)

    # [D, 136] fp16: cols 0..127 are A rows in (g, e, r) order, 128.. W_route;
    # re-packed to [128, KD, FE] so each partition's DMA line is contiguous.
    A_all = A.transpose(1, 0, 2, 3).reshape(F, D)
    AWT = np.concatenate([A_all.T, W_route.T], axis=1).astype(np.float16)
    AWT_dev = np.ascontiguousarray(
        AWT.reshape(KD, 128, FE).transpose(1, 0, 2))
    # block-diagonal B^T with SCALING folded in: rows (g,e,r), cols (g,o)
    BTbd = (Bw.transpose(1, 0, 3, 2).reshape(G, E * R, OD)
            * SCALING).astype(np.float16)

    if _nc_cache is None:
        _nc_cache = _build()
    nc = _nc_cache

    with ThreadPoolExecutor(NCORES) as ex:
        xTs = list(ex.map(lambda c: _shard_xT(x, c), range(NCORES)))
    in_maps = [{"xT": xTs[c], "AWT": AWT_dev, "BT": BTbd}
               for c in range(NCORES)]

    try:
        results = _run_cached(nc, in_maps)
    except Exception:  # noqa: BLE001  (fall back to the stock SPMD path)
        global _runner
        _runner = None
        res = run_bass_kernel_spmd(nc, in_maps, core_ids=list(range(NCORES)),
                                   **_RUN_KWARGS)
        results = res.results
    _LAST["results"] = results

    compact = np.concatenate(
        [results[c]["out"] for c in range(NCORES)], axis=0)
    outp = np.zeros((NTOK, OUT), dtype=np.float32)
    outp[:, lora_ind] = compact.astype(np.float32)
    return outp.reshape(B, S, OUT)


# revision 12
# speedup vs baseline: 1.0635x; 1.0053x over previous
"""MoELoRA forward kernel for 8x Trainium2 NeuronCores (Bass/Tile).

Math (see reference):
  route   = softmax(x @ W_route^T)                      [N, E]
  h       = x @ A[e,g,r,:]^T                            [N, E, G, R]
  wh      = h * route[..., None, None]
  compact = einsum(wh, Bw[e,g,o,r]) * SCALING           [N, G, OD]
  out     = zeros([N, OUT]); out[:, lora_ind] = compact.reshape(N, G*OD)

Device strategy (data-parallel over tokens, weights replicated):
  - Host pre-transposes each x shard to fp8-e3m4 xT [D, TPC]: the kernel is
    DMA-bandwidth-bound and x is the dominant input, so e3m4 (4 mantissa
    bits; empirically 1.35e-2 max rel err end-to-end vs the 2e-2 budget)
    halves the x read traffic. The contraction dim d lands on SBUF
    partitions with contiguous 512B DMA lines.
  - A is reordered to feature-major f = (g, e, r) and concatenated with
    W_route^T into one fp16 [128, KD, FE] rhs, pre-arranged on the host so
    each partition's DMA line is fully contiguous (2176B) for full-rate DMA.
    One accumulated matmul chain per 128-token tile produces h (cols
    0..127) and the routing logits (cols 128..135); fp8 lhsT x fp16 rhs is
    a legal mixed-precision matmul.
  - Softmax: exp (no max-subtract; logits are O(1)) with the row-sum fused
    into the ACT instruction via accum_out, one reciprocal, then
    probs = expv * rsum so the final PSUM->SBUF copies are scale-free and
    can run on any engine. SCALING=2 is folded into B on the host.
  - wh = h * probs uses a step-0 broadcast access pattern; wh is
    PE-transposed once per tile and the per-group up-projection runs as
    TWO K=128 matmuls of free-size 1024 against a block-diagonal fp16
    [128, 2048] B (fewer PE-SEQ instructions; PE.SEQ is near-critical).
  - The two [128,1024] fp32->fp16 PSUM drains go to Act and Pool (DVE
    keeps the softmax/wh chain), keeping every engine under the ~1.92us
    per-tile DMA cadence.
  - compact is staged fp16 in SBUF and DMAed out fp16 (halves the dominant
    write); the host upcasts and performs the lora_ind zero-pad scatter
    during unsharding.
"""

import sys
from concurrent.futures import ThreadPoolExecutor
from contextlib import ExitStack

for _p in ("/opt/trn_rl_repo", "/root/.axon_site/_ro/trn_rl_repo"):
    if _p not in sys.path:
        sys.path.insert(0, _p)

import ml_dtypes
import numpy as np

import concourse.bass as bass  # noqa: F401
import concourse.mybir as mybir
import concourse.tile as tile
from concourse import bacc
from concourse.bass_utils import run_bass_kernel_spmd
from concourse.masks import make_identity

# Problem dims (hardcoded per spec nn_MoELoRA_28089086116115)
B, S, D = 4, 4096, 1024
OUT = 3072
R, E, G = 8, 8, 2
OD = OUT // 3                    # 1024
F = G * E * R                    # 128 lora features, f = g*64 + e*8 + r
FE = F + E                       # 136: features + routing logits
SCALING = 16.0 / 8.0
NCORES = 8
NTOK = B * S                     # 16384
TPC = NTOK // NCORES             # 2048 tokens per core
TBLK = 512                       # tokens per x DMA block
NBLK = TPC // TBLK
KD = D // 128                    # 8 contraction chunks

F8 = ml_dtypes.float8_e3m4

# Hooks for test.py (not used by the grader, which calls kernel() only).
_RUN_KWARGS: dict = {}
_LAST: dict = {}

_nc_cache = None


def _build():
    f32 = mybir.dt.float32
    f16 = mybir.dt.float16
    f8 = mybir.dt.float8e3
    Exp = mybir.ActivationFunctionType.Exp
    Copy = mybir.ActivationFunctionType.Copy
    mult = mybir.AluOpType.mult

    nc = bacc.Bacc("TRN2", target_bir_lowering=False, debug=False,
                   num_devices=NCORES)
    xT = nc.dram_tensor("xT", [D, TPC], f8, kind="ExternalInput")
    awt = nc.dram_tensor("AWT", [128, KD, FE], f16, kind="ExternalInput")
    btbd = nc.dram_tensor("BT", [G, E * R, OD], f16, kind="ExternalInput")
    out = nc.dram_tensor("out", [TPC, G * OD], f16, kind="ExternalOutput")

    with tile.TileContext(nc) as tc, ExitStack() as ctx:
        wp = ctx.enter_context(tc.tile_pool(name="wp", bufs=1))
        awt_sb = wp.tile([128, KD, FE], f16)

        bt_sb = wp.tile([128, G * OD], f16)
        nc.gpsimd.memset(bt_sb[:], 0.0)
        ident = wp.tile([128, 128], f16)
        make_identity(nc, ident)

        # all x blocks live in SBUF at once (4 x 4KB/partition, fp8)
        xp = ctx.enter_context(tc.tile_pool(name="xp", bufs=NBLK))
        sp = ctx.enter_context(tc.tile_pool(name="sp", bufs=8))
        outp = ctx.enter_context(tc.tile_pool(name="outp", bufs=5))
        ph = ctx.enter_context(tc.tile_pool(name="ph", bufs=2, space="PSUM"))
        pt = ctx.enter_context(tc.tile_pool(name="pt", bufs=2, space="PSUM"))
        pc = ctx.enter_context(tc.tile_pool(name="pc", bufs=2, space="PSUM"))

        # weights first (compute needs awt + x block 0), then all x reads
        # up-front so no read ever queues behind a compute-gated write.
        nc.sync.dma_start(awt_sb[:], awt[:])
        x_sbs = []
        for blk in range(NBLK):
            x_sb = xp.tile([128, KD, TBLK], f8, name=f"x{blk}")
            xr = xT[:, blk * TBLK:(blk + 1) * TBLK].rearrange(
                "(k p) t -> p k t", p=128)
            if blk == 0:
                # split block 0 along k so the first h-matmuls start half a
                # block earlier (each k-line stays a full-rate 512B descriptor)
                nc.sync.dma_start(x_sb[:, 0:KD // 2, :], xr[:, 0:KD // 2, :])
                nc.sync.dma_start(x_sb[:, KD // 2:, :], xr[:, KD // 2:, :])
            else:
                nc.sync.dma_start(x_sb[:], xr)
            x_sbs.append(x_sb)
            if blk == 0:
                # BT is block-diagonal: zero the tile (idle Pool engine) and
                # DMA only the two nonzero 128KB blocks.
                nc.sync.dma_start(bt_sb[0:64, 0:1024], btbd[0])
                nc.sync.dma_start(bt_sb[64:128, 1024:2048], btbd[1])

        # Two-stage software pipeline over the 16 128-token subtiles: iter N
        # runs the h-matmuls + softmax/wh for subtile N while transposing,
        # up-projecting and draining subtile N-1. This keeps every engine's
        # in-order queue free of head-of-line stalls (the drains land after
        # the same-iteration up-proj matmuls, the transpose input is a full
        # iteration old) so the steady-state cadence is DMA-paced.
        NSUB = TPC // 128
        wh_t = [None] * NSUB       # wh tiles (SBUF fp16), stage S2 output
        o_sbs = [None] * (NSUB // 2)

        for N in range(NSUB + 1):
            M = N - 1

            # S3 first: the transpose+copy of the PREVIOUS subtile's wh lead
            # both the PE and DVE queues, so the up-proj's operand is ready
            # before PE reaches it — the loop-carried path stays short and
            # this iteration's softmax chain has a full iteration to finish.
            if 0 <= M < NSUB:
                whT_ps = pt.tile([128, 128], f16)
                nc.tensor.transpose(whT_ps[:], wh_t[M][:], ident[:])
                whT = sp.tile([128, 128], f16)
                nc.vector.tensor_copy(whT[:], whT_ps[:])

            if N < NSUB:
                x_sb = x_sbs[N // 4]
                t0 = (N % 4) * 128
                # S1: h (cols 0..127) + routing logits (cols 128..135)
                hE = ph.tile([128, FE], f32)
                for k in range(KD):
                    nc.tensor.matmul(
                        hE[:],
                        lhsT=x_sb[:, k, t0:t0 + 128],
                        rhs=awt_sb[:, k, :],
                        start=(k == 0),
                        stop=(k == KD - 1),
                    )
                # S2: softmax probs = exp(logits) / sum, then wh = h * probs
                expv = sp.tile([128, E], f32)
                ssum = sp.tile([128, 1], f32)
                # plain exp (no accum_out: the accumulator read costs Act an
                # extra 187ns and Act is the tightest engine); sum on DVE
                nc.scalar.activation(expv[:], hE[:, F:FE], Exp)
                nc.vector.reduce_sum(ssum[:], expv[:],
                                     axis=mybir.AxisListType.X)
                rsum = sp.tile([128, 1], f32)
                nc.vector.reciprocal(rsum[:], ssum[:])
                probs = sp.tile([128, E], f32)
                nc.gpsimd.tensor_scalar_mul(probs[:], expv[:], rsum[:, 0:1])
                wh = sp.tile([128, F], f16)
                nc.vector.tensor_tensor(
                    out=wh.rearrange("p (g e r) -> p g e r", g=G, e=E),
                    in0=hE[:, 0:F].rearrange("p (g e r) -> p g e r", g=G, e=E),
                    in1=probs[:, None, :, None].to_broadcast([128, G, E, R]),
                    op=mult,
                )
                wh_t[N] = wh

            if 0 <= M < NSUB:
                half = M % 2
                if half == 0:
                    o_sbs[M // 2] = outp.tile([128, 2, G * OD], f16,
                                              name=f"o{M // 2}", tag="o")
                o_sb = o_sbs[M // 2]
                # S4: compact[t, (g,o)] via block-diagonal 2*B^T (K=128),
                # one PSUM bank per 512-col matmul
                cps_l = []
                for j in range(4):
                    cps = pc.tile([128, 512], f32, name=f"cps{j}", tag="cps")
                    nc.tensor.matmul(
                        cps[:],
                        lhsT=whT[:],
                        rhs=bt_sb[:, j * 512:(j + 1) * 512],
                        start=True,
                        stop=True,
                    )
                    cps_l.append(cps)
                # S5: plain fp32->fp16 PSUM drains (gpsimd cannot read PSUM
                # on this target): Act gets 2.5 per subtile, DVE 1.5
                for j in range(4):
                    dst = o_sb[:, half, j * 512:(j + 1) * 512]
                    on_act = (j in (0, 2)) or (j == 3 and M % 2 == 0)
                    if on_act:
                        nc.scalar.activation(dst, cps_l[j][:], Copy)
                    else:
                        nc.vector.tensor_copy(dst, cps_l[j][:])
                # S6: output writes. Edge pairs go out per-subtile (the first
                # writes start a subtile earlier; the final write is half as
                # long); steady-state pairs share one 1 MiB write.
                pair = M // 2
                edge = pair <= 2 or pair >= NSUB // 2 - 2
                r0 = pair * 256
                if edge:
                    nc.sync.dma_start(
                        out[r0 + half * 128:r0 + (half + 1) * 128, :],
                        o_sb[:, half, :])
                elif half == 1:
                    nc.sync.dma_start(
                        out[r0:r0 + 256, :].rearrange(
                            "(s p) o -> p s o", p=128),
                        o_sb[:])

    nc.compile()
    return nc


def _shard_xT(x, c):
    return np.ascontiguousarray(x[c * TPC:(c + 1) * TPC].T).astype(F8)


_runner = None


def _get_runner(nc):
    """Build the sharded PJRT callable once; reuse across kernel() calls.

    Mirrors bass2jax.run_bass_via_pjrt's multi-core branch, but caches the
    jitted function so repeat calls skip retrace/recompile. Falls back to
    the stock path (handled by caller) on any failure.
    """
    global _runner
    if _runner is not None:
        return _runner
    import jax
    from jax.experimental.shard_map import shard_map
    from jax.sharding import Mesh, PartitionSpec

    from concourse import bass2jax, mybir as _mb

    bass2jax.install_neuronx_cc_hook()
    partition_name = (nc.partition_id_tensor.name
                      if nc.partition_id_tensor else None)
    in_names, out_names, out_avals = [], [], []
    for alloc in nc.m.functions[0].allocations:
        if not isinstance(alloc, _mb.MemoryLocationSet):
            continue
        name = alloc.memorylocations[0].name
        if alloc.kind == "ExternalInput":
            if name != partition_name:
                in_names.append(name)
        elif alloc.kind == "ExternalOutput":
            out_names.append(name)
            out_avals.append(jax.core.ShapedArray(
                tuple(alloc.tensor_shape), _mb.dt.np(alloc.dtype)))
    n_params = len(in_names)
    n_outs = len(out_avals)
    all_in_names = list(in_names) + list(out_names)
    if partition_name is not None:
        all_in_names.append(partition_name)

    def _body(*args):
        operands = list(args)
        if partition_name is not None:
            operands.append(bass2jax.partition_id_tensor())
        outs = bass2jax._bass_exec_p.bind(
            *operands,
            out_avals=tuple(out_avals),
            in_names=tuple(all_in_names),
            out_names=tuple(out_names),
            lowering_input_output_aliases=(),
            sim_require_finite=True,
            sim_require_nnan=True,
            nc=nc,
        )
        return tuple(outs)

    devices = jax.devices()[:NCORES]
    mesh = Mesh(np.asarray(devices), ("core",))
    specs = (PartitionSpec("core"),) * (n_params + n_outs)
    sharded = jax.jit(
        shard_map(_body, mesh=mesh, in_specs=specs,
                  out_specs=(PartitionSpec("core"),) * n_outs,
                  check_rep=False),
        donate_argnums=tuple(range(n_params, n_params + n_outs)),
        keep_unused=True,
    )
    _runner = (sharded, in_names, out_names, out_avals)
    return _runner


def _run_cached(nc, in_maps):
    sharded, in_names, out_names, out_avals = _get_runner(nc)
    concat_in = [
        np.concatenate([np.asarray(m[name]) for m in in_maps], axis=0)
        for name in in_names
    ]
    concat_zeros = [
        np.zeros((NCORES * a.shape[0], *a.shape[1:]), a.dtype)
        for a in out_avals
    ]
    out_arrs = sharded(*concat_in, *concat_zeros)
    return [
        {name: np.asarray(out_arrs[i]).reshape(NCORES, *out_avals[i].shape)[c]
         for i, name in enumerate(out_names)}
        for c in range(NCORES)
    ]


def kernel(x, W_route, A, Bw, lora_ind):
    global _nc_cache
    x = np.asarray(x, dtype=np.float32).reshape(NTOK, D)
    W_route = np.asarray(W_route, dtype=np.float32)
    A = np.asarray(A, dtype=np.float32)
    Bw = np.asarray(Bw, dtype=np.float32)
    lora_ind = np.asarray(lora_ind).astype(np.int64)

    # [D, 136] fp16: cols 0..127 are A rows in (g, e, r) order, 128.. W_route;
    # re-packed to [128, KD, FE] so each partition's DMA line is contiguous.
    A_all = A.transpose(1, 0, 2, 3).reshape(F, D)
    AWT = np.concatenate([A_all.T, W_route.T], axis=1).astype(np.float16)
    AWT_dev = np.ascontiguousarray(
        AWT.reshape(KD, 128, FE).transpose(1, 0, 2))
    # block-diagonal B^T with SCALING folded in: rows (g,e,r), cols (g,o)
    BTbd = (Bw.transpose(1, 0, 3, 2).reshape(G, E * R, OD)
            * SCALING).astype(np.float16)

    if _nc_cache is None:
        _nc_cache = _build()
    nc = _nc_cache

    with ThreadPoolExecutor(NCORES) as ex:
        xTs = list(ex.map(lambda c: _shard_xT(x, c), range(NCORES)))
    in_maps = [{"xT": xTs[c], "AWT": AWT_dev, "BT": BTbd}
               for c in range(NCORES)]

    try:
        results = _run_cached(nc, in_maps)
    except Exception:  # noqa: BLE001  (fall back to the stock SPMD path)
        global _runner
        _runner = None
        res = run_bass_kernel_spmd(nc, in_maps, core_ids=list(range(NCORES)),
                                   **_RUN_KWARGS)
        results = res.results
    _LAST["results"] = results

    compact = np.concatenate(
        [results[c]["out"] for c in range(NCORES)], axis=0)
    outp = np.zeros((NTOK, OUT), dtype=np.float32)
    outp[:, lora_ind] = compact.astype(np.float32)
    return outp.reshape(B, S, OUT)


# revision 15
# speedup vs baseline: 1.0766x; 1.0123x over previous
"""MoELoRA forward kernel for 8x Trainium2 NeuronCores (Bass/Tile).

Math (see reference):
  route   = softmax(x @ W_route^T)                      [N, E]
  h       = x @ A[e,g,r,:]^T                            [N, E, G, R]
  wh      = h * route[..., None, None]
  compact = einsum(wh, Bw[e,g,o,r]) * SCALING           [N, G, OD]
  out     = zeros([N, OUT]); out[:, lora_ind] = compact.reshape(N, G*OD)

Device strategy (data-parallel over tokens, weights replicated):
  - Host pre-transposes each x shard to fp8-e3m4 xT [D, TPC]: the kernel is
    DMA-bandwidth-bound and x is the dominant input, so e3m4 (4 mantissa
    bits; empirically 1.35e-2 max rel err end-to-end vs the 2e-2 budget)
    halves the x read traffic. The contraction dim d lands on SBUF
    partitions with contiguous 512B DMA lines.
  - A is reordered to feature-major f = (g, e, r) and concatenated with
    W_route^T into one fp16 [128, KD, FE] rhs, pre-arranged on the host so
    each partition's DMA line is fully contiguous (2176B) for full-rate DMA.
    One accumulated matmul chain per 128-token tile produces h (cols
    0..127) and the routing logits (cols 128..135); fp8 lhsT x fp16 rhs is
    a legal mixed-precision matmul.
  - Softmax: exp (no max-subtract; logits are O(1)) with the row-sum fused
    into the ACT instruction via accum_out, one reciprocal, then
    probs = expv * rsum so the final PSUM->SBUF copies are scale-free and
    can run on any engine. SCALING=2 is folded into B on the host.
  - wh = h * probs uses a step-0 broadcast access pattern; wh is
    PE-transposed once per tile and the per-group up-projection runs as
    TWO K=128 matmuls of free-size 1024 against a block-diagonal fp16
    [128, 2048] B (fewer PE-SEQ instructions; PE.SEQ is near-critical).
  - The two [128,1024] fp32->fp16 PSUM drains go to Act and Pool (DVE
    keeps the softmax/wh chain), keeping every engine under the ~1.92us
    per-tile DMA cadence.
  - compact is staged fp16 in SBUF and DMAed out fp16 (halves the dominant
    write); the host upcasts and performs the lora_ind zero-pad scatter
    during unsharding.
"""

import sys
from concurrent.futures import ThreadPoolExecutor
from contextlib import ExitStack

for _p in ("/opt/trn_rl_repo", "/root/.axon_site/_ro/trn_rl_repo"):
    if _p not in sys.path:
        sys.path.insert(0, _p)

import ml_dtypes
import numpy as np

import concourse.bass as bass  # noqa: F401
import concourse.mybir as mybir
import concourse.tile as tile
from concourse import bacc
from concourse.bass_utils import run_bass_kernel_spmd
from concourse.masks import make_identity

# Problem dims (hardcoded per spec nn_MoELoRA_28089086116115)
B, S, D = 4, 4096, 1024
OUT = 3072
R, E, G = 8, 8, 2
OD = OUT // 3                    # 1024
F = G * E * R                    # 128 lora features, f = g*64 + e*8 + r
FE = F + E                       # 136: features + routing logits
SCALING = 16.0 / 8.0
NCORES = 8
NTOK = B * S                     # 16384
TPC = NTOK // NCORES             # 2048 tokens per core
TBLK = 512                       # tokens per x DMA block
NBLK = TPC // TBLK
KD = D // 128                    # 8 contraction chunks

F8 = ml_dtypes.float8_e3m4

# Hooks for test.py (not used by the grader, which calls kernel() only).
_RUN_KWARGS: dict = {}
_LAST: dict = {}

_nc_cache = None


def _build():
    f32 = mybir.dt.float32
    f16 = mybir.dt.float16
    f8 = mybir.dt.float8e3
    Exp = mybir.ActivationFunctionType.Exp
    Copy = mybir.ActivationFunctionType.Copy
    mult = mybir.AluOpType.mult

    nc = bacc.Bacc("TRN2", target_bir_lowering=False, debug=False,
                   num_devices=NCORES)
    xT = nc.dram_tensor("xT", [D, TPC], f8, kind="ExternalInput")
    awt = nc.dram_tensor("AWT", [128, KD, FE], f16, kind="ExternalInput")
    btbd = nc.dram_tensor("BT", [G, E * R, OD], f16, kind="ExternalInput")
    out = nc.dram_tensor("out", [TPC, G * OD], f16, kind="ExternalOutput")

    with tile.TileContext(nc) as tc, ExitStack() as ctx:
        wp = ctx.enter_context(tc.tile_pool(name="wp", bufs=1))
        awt_sb = wp.tile([128, KD, FE], f16)

        bt_sb = wp.tile([128, G * OD], f16)
        nc.gpsimd.memset(bt_sb[:], 0.0)
        ident = wp.tile([128, 128], f16)
        make_identity(nc, ident)

        # all x blocks live in SBUF at once (4 x 4KB/partition, fp8)
        xp = ctx.enter_context(tc.tile_pool(name="xp", bufs=NBLK))
        sp = ctx.enter_context(tc.tile_pool(name="sp", bufs=8))
        outp = ctx.enter_context(tc.tile_pool(name="outp", bufs=5))
        ph = ctx.enter_context(tc.tile_pool(name="ph", bufs=2, space="PSUM"))
        pt = ctx.enter_context(tc.tile_pool(name="pt", bufs=2, space="PSUM"))
        pc = ctx.enter_context(tc.tile_pool(name="pc", bufs=4, space="PSUM"))

        # weights first (compute needs awt + x block 0), then all x reads
        # up-front so no read ever queues behind a compute-gated write.
        nc.sync.dma_start(awt_sb[:], awt[:])
        x_sbs = []
        for blk in range(NBLK):
            x_sb = xp.tile([128, KD, TBLK], f8, name=f"x{blk}")
            xr = xT[:, blk * TBLK:(blk + 1) * TBLK].rearrange(
                "(k p) t -> p k t", p=128)
            if blk == 0:
                # split block 0 along k so the first h-matmuls start half a
                # block earlier (each k-line stays a full-rate 512B descriptor)
                nc.sync.dma_start(x_sb[:, 0:KD // 2, :], xr[:, 0:KD // 2, :])
                nc.sync.dma_start(x_sb[:, KD // 2:, :], xr[:, KD // 2:, :])
            else:
                nc.sync.dma_start(x_sb[:], xr)
            x_sbs.append(x_sb)
            if blk == 0:
                # BT is block-diagonal: zero the tile (idle Pool engine) and
                # DMA only the two nonzero 128KB blocks.
                nc.sync.dma_start(bt_sb[0:64, 0:1024], btbd[0])
                nc.sync.dma_start(bt_sb[64:128, 1024:2048], btbd[1])

        # Two-stage software pipeline over the 16 128-token subtiles: iter N
        # runs the h-matmuls + softmax/wh for subtile N while transposing,
        # up-projecting and draining subtile N-1. This keeps every engine's
        # in-order queue free of head-of-line stalls (the drains land after
        # the same-iteration up-proj matmuls, the transpose input is a full
        # iteration old) so the steady-state cadence is DMA-paced.
        NSUB = TPC // 128
        wh_t = [None] * NSUB       # wh tiles (SBUF fp16), stage S2 output
        o_sbs = [None] * (NSUB // 2)

        for N in range(NSUB + 1):
            M = N - 1

            # S3 first: the transpose+copy of the PREVIOUS subtile's wh lead
            # both the PE and DVE queues, so the up-proj's operand is ready
            # before PE reaches it — the loop-carried path stays short and
            # this iteration's softmax chain has a full iteration to finish.
            if 0 <= M < NSUB:
                whT_ps = pt.tile([128, 128], f16)
                nc.tensor.transpose(whT_ps[:], wh_t[M][:], ident[:])
                whT = sp.tile([128, 128], f16, tag="whT")
                nc.vector.tensor_copy(whT[:], whT_ps[:])

            if N < NSUB:
                x_sb = x_sbs[N // 4]
                t0 = (N % 4) * 128
                # S1: h (cols 0..127) + routing logits (cols 128..135)
                hE = ph.tile([128, FE], f32)
                for k in range(KD):
                    nc.tensor.matmul(
                        hE[:],
                        lhsT=x_sb[:, k, t0:t0 + 128],
                        rhs=awt_sb[:, k, :],
                        start=(k == 0),
                        stop=(k == KD - 1),
                    )
                # S2: softmax probs = exp(logits) / sum, then wh = h * probs
                expv = sp.tile([128, E], f32, tag="expv")
                ssum = sp.tile([128, 1], f32, tag="ssum")
                # plain exp (no accum_out: the accumulator read costs Act an
                # extra 187ns and Act is the tightest engine); sum on DVE
                nc.scalar.activation(expv[:], hE[:, F:FE], Exp)
                nc.vector.reduce_sum(ssum[:], expv[:],
                                     axis=mybir.AxisListType.X)
                rsum = sp.tile([128, 1], f32, tag="rsum")
                nc.vector.reciprocal(rsum[:], ssum[:])
                probs = sp.tile([128, E], f32, tag="probs")
                nc.gpsimd.tensor_scalar_mul(probs[:], expv[:], rsum[:, 0:1])
                wh = sp.tile([128, F], f16, tag="wh")
                nc.vector.tensor_tensor(
                    out=wh.rearrange("p (g e r) -> p g e r", g=G, e=E),
                    in0=hE[:, 0:F].rearrange("p (g e r) -> p g e r", g=G, e=E),
                    in1=probs[:, None, :, None].to_broadcast([128, G, E, R]),
                    op=mult,
                )
                wh_t[N] = wh

            if 0 <= M < NSUB:
                half = M % 2
                if half == 0:
                    o_sbs[M // 2] = outp.tile([128, 2, G * OD], f16,
                                              name=f"o{M // 2}", tag="o")
                o_sb = o_sbs[M // 2]
                # S4: compact[t, (g,o)] via block-diagonal 2*B^T (K=128),
                # one PSUM bank per 512-col matmul
                cps_l = []
                for j in range(4):
                    cps = pc.tile([128, 512], f32, name=f"cps{j}", tag="cps")
                    nc.tensor.matmul(
                        cps[:],
                        lhsT=whT[:],
                        rhs=bt_sb[:, j * 512:(j + 1) * 512],
                        start=True,
                        stop=True,
                    )
                    cps_l.append(cps)
                # S5: plain fp32->fp16 PSUM drains (gpsimd cannot read PSUM
                # on this target): Act gets 2.5 per subtile, DVE 1.5
                for j in range(4):
                    dst = o_sb[:, half, j * 512:(j + 1) * 512]
                    on_act = (j in (0, 2)) or (j == 3 and M % 2 == 0)
                    if on_act:
                        nc.scalar.activation(dst, cps_l[j][:], Copy)
                    else:
                        nc.vector.tensor_copy(dst, cps_l[j][:])
                # S6: output writes. Edge pairs go out per-subtile (the first
                # writes start a subtile earlier; the final write is half as
                # long); steady-state pairs share one 1 MiB write.
                pair = M // 2
                edge = pair <= 2 or pair >= NSUB // 2 - 2
                r0 = pair * 256
                if edge:
                    nc.sync.dma_start(
                        out[r0 + half * 128:r0 + (half + 1) * 128, :],
                        o_sb[:, half, :])
                elif half == 1:
                    nc.sync.dma_start(
                        out[r0:r0 + 256, :].rearrange(
                            "(s p) o -> p s o", p=128),
                        o_sb[:])

    nc.compile()
    return nc


def _shard_xT(x, c):
    return np.ascontiguousarray(x[c * TPC:(c + 1) * TPC].T).astype(F8)


_runner = None


def _get_runner(nc):
    """Build the sharded PJRT callable once; reuse across kernel() calls.

    Mirrors bass2jax.run_bass_via_pjrt's multi-core branch, but caches the
    jitted function so repeat calls skip retrace/recompile. Falls back to
    the stock path (handled by caller) on any failure.
    """
    global _runner
    if _runner is not None:
        return _runner
    import jax
    from jax.experimental.shard_map import shard_map
    from jax.sharding import Mesh, PartitionSpec

    from concourse import bass2jax, mybir as _mb

    bass2jax.install_neuronx_cc_hook()
    partition_name = (nc.partition_id_tensor.name
                      if nc.partition_id_tensor else None)
    in_names, out_names, out_avals = [], [], []
    for alloc in nc.m.functions[0].allocations:
        if not isinstance(alloc, _mb.MemoryLocationSet):
            continue
        name = alloc.memorylocations[0].name
        if alloc.kind == "ExternalInput":
            if name != partition_name:
                in_names.append(name)
        elif alloc.kind == "ExternalOutput":
            out_names.append(name)
            out_avals.append(jax.core.ShapedArray(
                tuple(alloc.tensor_shape), _mb.dt.np(alloc.dtype)))
    n_params = len(in_names)
    n_outs = len(out_avals)
    all_in_names = list(in_names) + list(out_names)
    if partition_name is not None:
        all_in_names.append(partition_name)

    def _body(*args):
        operands = list(args)
        if partition_name is not None:
            operands.append(bass2jax.partition_id_tensor())
        outs = bass2jax._bass_exec_p.bind(
            *operands,
            out_avals=tuple(out_avals),
            in_names=tuple(all_in_names),
            out_names=tuple(out_names),
            lowering_input_output_aliases=(),
            sim_require_finite=True,
            sim_require_nnan=True,
            nc=nc,
        )
        return tuple(outs)

    devices = jax.devices()[:NCORES]
    mesh = Mesh(np.asarray(devices), ("core",))
    specs = (PartitionSpec("core"),) * (n_params + n_outs)
    sharded = jax.jit(
        shard_map(_body, mesh=mesh, in_specs=specs,
                  out_specs=(PartitionSpec("core"),) * n_outs,
                  check_rep=False),
        donate_argnums=tuple(range(n_params, n_params + n_outs)),
        keep_unused=True,
    )
    _runner = (sharded, in_names, out_names, out_avals)
    return _runner


def _run_cached(nc, in_maps):
    sharded, in_names, out_names, out_avals = _get_runner(nc)
    concat_in = [
        np.concatenate([np.asarray(m[name]) for m in in_maps], axis=0)
        for name in in_names
    ]
    concat_zeros = [
        np.zeros((NCORES * a.shape[0], *a.shape[1:]), a.dtype)
        for a in out_avals
    ]
    out_arrs = sharded(*concat_in, *concat_zeros)
    return [
        {name: np.asarray(out_arrs[i]).reshape(NCORES, *out_avals[i].shape)[c]
         for i, name in enumerate(out_names)}
        for c in range(NCORES)
    ]


def kernel(x, W_route, A, Bw, lora_ind):
    global _nc_cache
    x = np.asarray(x, dtype=np.float32).reshape(NTOK, D)
    W_route = np.asarray(W_route, dtype=np.float32)
    A = np.asarray(A, dtype=np.float32)
    Bw = np.asarray(Bw, dtype=np.float32)
    lora_ind = np.asarray(lora_ind).astype(np.int64)

    # [D, 136] fp16: cols 0..127 are A rows in (g, e, r) order, 128.. W_route;
    # re-packed to [128, KD, FE] so each partition's DMA line is contiguous.
    A_all = A.transpose(1, 0, 2, 3).reshape(F, D)
    AWT = np.concatenate([A_all.T, W_route.T], axis=1).astype(np.float16)
    AWT_dev = np.ascontiguousarray(
        AWT.reshape(KD, 128, FE).transpose(1, 0, 2))
    # block-diagonal B^T with SCALING folded in: rows (g,e,r), cols (g,o)
    BTbd = (Bw.transpose(1, 0, 3, 2).reshape(G, E * R, OD)
            * SCALING).astype(np.float16)

    if _nc_cache is None:
        _nc_cache = _build()
    nc = _nc_cache

    with ThreadPoolExecutor(NCORES) as ex:
        xTs = list(ex.map(lambda c: _shard_xT(x, c), range(NCORES)))
    in_maps = [{"xT": xTs[c], "AWT": AWT_dev, "BT": BTbd}
               for c in range(NCORES)]

    try:
        results = _run_cached(nc, in_maps)
    except Exception:  # noqa: BLE001  (fall back to the stock SPMD path)
        global _runner
        _runner = None
        res = run_bass_kernel_spmd(nc, in_maps, core_ids=list(range(NCORES)),
                                   **_RUN_KWARGS)
        results = res.results
    _LAST["results"] = results

    compact = np.concatenate(
        [results[c]["out"] for c in range(NCORES)], axis=0)
    outp = np.zeros((NTOK, OUT), dtype=np.float32)
    outp[:, lora_ind] = compact.astype(np.float32)
    return outp.reshape(B, S, OUT)


# revision 18
# speedup vs baseline: 1.2365x; 1.1486x over previous
"""MoELoRA forward kernel for 8x Trainium2 NeuronCores (Bass/Tile).

Math (see reference):
  route   = softmax(x @ W_route^T)                      [N, E]
  h       = x @ A[e,g,r,:]^T                            [N, E, G, R]
  wh      = h * route[..., None, None]
  compact = einsum(wh, Bw[e,g,o,r]) * SCALING           [N, G, OD]
  out     = zeros([N, OUT]); out[:, lora_ind] = compact.reshape(N, G*OD)

Device strategy (data-parallel over tokens, weights replicated):
  - Host pre-transposes each x shard to fp8-e3m4 xT [D, TPC]: the kernel is
    DMA-bandwidth-bound and x is the dominant input, so e3m4 (4 mantissa
    bits; empirically 1.35e-2 max rel err end-to-end vs the 2e-2 budget)
    halves the x read traffic. The contraction dim d lands on SBUF
    partitions with contiguous 512B DMA lines.
  - A is reordered to feature-major f = (g, e, r) and concatenated with
    W_route^T into one fp16 [128, KD, FE] rhs, pre-arranged on the host so
    each partition's DMA line is fully contiguous (2176B) for full-rate DMA.
    One accumulated matmul chain per 128-token tile produces h (cols
    0..127) and the routing logits (cols 128..135); fp8 lhsT x fp16 rhs is
    a legal mixed-precision matmul.
  - Softmax: exp (no max-subtract; logits are O(1)) with the row-sum fused
    into the ACT instruction via accum_out, one reciprocal, then
    probs = expv * rsum so the final PSUM->SBUF copies are scale-free and
    can run on any engine. SCALING=2 is folded into B on the host.
  - wh = h * probs uses a step-0 broadcast access pattern; wh is
    PE-transposed once per tile and the per-group up-projection runs as
    TWO K=128 matmuls of free-size 1024 against a block-diagonal fp16
    [128, 2048] B (fewer PE-SEQ instructions; PE.SEQ is near-critical).
  - The two [128,1024] fp32->fp16 PSUM drains go to Act and Pool (DVE
    keeps the softmax/wh chain), keeping every engine under the ~1.92us
    per-tile DMA cadence.
  - compact is staged fp16 in SBUF and DMAed out fp16 (halves the dominant
    write); the host upcasts and performs the lora_ind zero-pad scatter
    during unsharding.
"""

import sys
from concurrent.futures import ThreadPoolExecutor
from contextlib import ExitStack

for _p in ("/opt/trn_rl_repo", "/root/.axon_site/_ro/trn_rl_repo"):
    if _p not in sys.path:
        sys.path.insert(0, _p)

import ml_dtypes
import numpy as np

import concourse.bass as bass  # noqa: F401
import concourse.mybir as mybir
import concourse.tile as tile
from concourse import bacc
from concourse.bass_utils import run_bass_kernel_spmd
from concourse.masks import make_identity

# Problem dims (hardcoded per spec nn_MoELoRA_28089086116115)
B, S, D = 4, 4096, 1024
OUT = 3072
R, E, G = 8, 8, 2
OD = OUT // 3                    # 1024
F = G * E * R                    # 128 lora features, f = g*64 + e*8 + r
FE = F + E                       # 136: features + routing logits
SCALING = 16.0 / 8.0
NCORES = 8
NTOK = B * S                     # 16384
TPC = NTOK // NCORES             # 2048 tokens per core
TBLK = 512                       # tokens per x DMA block
NBLK = TPC // TBLK
KD = D // 128                    # 8 contraction chunks

F8 = ml_dtypes.float8_e3m4

# Hooks for test.py (not used by the grader, which calls kernel() only).
_RUN_KWARGS: dict = {}
_LAST: dict = {}

_nc_cache = None


def _build():
    f32 = mybir.dt.float32
    f16 = mybir.dt.float16
    f8 = mybir.dt.float8e3
    Exp = mybir.ActivationFunctionType.Exp
    Copy = mybir.ActivationFunctionType.Copy
    mult = mybir.AluOpType.mult

    nc = bacc.Bacc("TRN2", target_bir_lowering=False, debug=False,
                   num_devices=NCORES)
    xT = nc.dram_tensor("xT", [D, TPC], f8, kind="ExternalInput")
    awt = nc.dram_tensor("AWT", [128, KD, FE], f16, kind="ExternalInput")
    btbd = nc.dram_tensor("BT", [G, E * R, OD], f16, kind="ExternalInput")
    out = nc.dram_tensor("out", [TPC, G * OD], f16, kind="ExternalOutput")

    with tile.TileContext(nc) as tc, ExitStack() as ctx:
        wp = ctx.enter_context(tc.tile_pool(name="wp", bufs=1))
        awt_sb = wp.tile([128, KD, FE], f16)

        bt_sb = wp.tile([128, G * OD], f16)
        nc.gpsimd.memset(bt_sb[:], 0.0)
        ident = wp.tile([128, 128], f16)
        make_identity(nc, ident)

        # all x blocks live in SBUF at once (4 x 4KB/partition, fp8)
        xp = ctx.enter_context(tc.tile_pool(name="xp", bufs=NBLK))
        sp = ctx.enter_context(tc.tile_pool(name="sp", bufs=8))
        outp = ctx.enter_context(tc.tile_pool(name="outp", bufs=5))
        ph = ctx.enter_context(tc.tile_pool(name="ph", bufs=2, space="PSUM"))
        pt = ctx.enter_context(tc.tile_pool(name="pt", bufs=2, space="PSUM"))
        pc = ctx.enter_context(tc.tile_pool(name="pc", bufs=4, space="PSUM"))

        # weights first (compute needs awt + x block 0), then all x reads
        # up-front so no read ever queues behind a compute-gated write.
        nc.sync.dma_start(awt_sb[:], awt[:])
        x_sbs = []
        for blk in range(NBLK):
            x_sb = xp.tile([128, KD, TBLK], f8, name=f"x{blk}")
            xr = xT[:, blk * TBLK:(blk + 1) * TBLK].rearrange(
                "(k p) t -> p k t", p=128)
            if blk == 0:
                # split block 0 along k so the first h-matmuls start half a
                # block earlier (each k-line stays a full-rate 512B descriptor)
                nc.sync.dma_start(x_sb[:, 0:KD // 2, :], xr[:, 0:KD // 2, :])
                nc.sync.dma_start(x_sb[:, KD // 2:, :], xr[:, KD // 2:, :])
            else:
                nc.sync.dma_start(x_sb[:], xr)
            x_sbs.append(x_sb)
            if blk == 0:
                # BT is block-diagonal: zero the tile (idle Pool engine) and
                # DMA only the two nonzero 128KB blocks.
                nc.sync.dma_start(bt_sb[0:64, 0:1024], btbd[0])
                nc.sync.dma_start(bt_sb[64:128, 1024:2048], btbd[1])

        # Two-stage software pipeline over the 16 128-token subtiles: iter N
        # runs the h-matmuls + softmax/wh for subtile N while transposing,
        # up-projecting and draining subtile N-1. This keeps every engine's
        # in-order queue free of head-of-line stalls (the drains land after
        # the same-iteration up-proj matmuls, the transpose input is a full
        # iteration old) so the steady-state cadence is DMA-paced.
        NSUB = TPC // 128
        wh_t = [None] * NSUB       # wh tiles (SBUF fp16), stage S2 output
        o_sbs = [None] * (NSUB // 2)

        # Iteration N transposes/up-projects/drains subtile N while running
        # the h-matmuls + softmax for subtile N+2. The two-iteration lead
        # means the transpose's input (TT of subtile N, computed in iter
        # N-2) has ~2 full periods of slack, so the in-order Act queue's
        # drain bursts never delay the loop-carried chain.
        for N in range(-2, NSUB):
            # S3: transpose + SBUF copy of subtile N's wh lead the PE and
            # DVE queues so the up-proj operand is ready before PE needs it.
            if N >= 0:
                whT_ps = pt.tile([128, 128], f16)
                nc.tensor.transpose(whT_ps[:], wh_t[N][:], ident[:])
                whT = sp.tile([128, 128], f16, tag="whT")
                nc.vector.tensor_copy(whT[:], whT_ps[:])

            K = N + 2
            if K < NSUB:
                x_sb = x_sbs[K // 4]
                t0 = (K % 4) * 128
                # S1: h (cols 0..127) + routing logits (cols 128..135)
                hE = ph.tile([128, FE], f32)
                for k in range(KD):
                    nc.tensor.matmul(
                        hE[:],
                        lhsT=x_sb[:, k, t0:t0 + 128],
                        rhs=awt_sb[:, k, :],
                        start=(k == 0),
                        stop=(k == KD - 1),
                    )
                # S2: softmax probs = exp(logits) / sum, then wh = h * probs
                expv = sp.tile([128, E], f32, tag="expv")
                ssum = sp.tile([128, 1], f32, tag="ssum")
                # plain exp (no accum_out: the accumulator read costs Act an
                # extra 187ns and Act is the tightest engine); sum on DVE
                nc.scalar.activation(expv[:], hE[:, F:FE], Exp)
                nc.vector.reduce_sum(ssum[:], expv[:],
                                     axis=mybir.AxisListType.X)
                rsum = sp.tile([128, 1], f32, tag="rsum")
                nc.vector.reciprocal(rsum[:], ssum[:])
                probs = sp.tile([128, E], f32, tag="probs")
                nc.gpsimd.tensor_scalar_mul(probs[:], expv[:], rsum[:, 0:1])
                wh = sp.tile([128, F], f16, tag="wh")
                nc.vector.tensor_tensor(
                    out=wh.rearrange("p (g e r) -> p g e r", g=G, e=E),
                    in0=hE[:, 0:F].rearrange("p (g e r) -> p g e r", g=G, e=E),
                    in1=probs[:, None, :, None].to_broadcast([128, G, E, R]),
                    op=mult,
                )
                wh_t[K] = wh

            if N >= 0:
                half = N % 2
                if half == 0:
                    o_sbs[N // 2] = outp.tile([128, 2, G * OD], f16,
                                              name=f"o{N // 2}", tag="o")
                o_sb = o_sbs[N // 2]
                # S4: compact[t, (g,o)] via block-diagonal 2*B^T (K=128),
                # one PSUM bank per 512-col matmul
                cps_l = []
                for j in range(4):
                    cps = pc.tile([128, 512], f32, name=f"cps{j}", tag="cps")
                    nc.tensor.matmul(
                        cps[:],
                        lhsT=whT[:],
                        rhs=bt_sb[:, j * 512:(j + 1) * 512],
                        start=True,
                        stop=True,
                    )
                    cps_l.append(cps)
                # S5: plain fp32->fp16 PSUM drains (gpsimd cannot read PSUM
                # on this target): Act gets 2.5 per subtile, DVE 1.5
                for j in range(4):
                    dst = o_sb[:, half, j * 512:(j + 1) * 512]
                    on_act = (j in (0, 2)) or (j == 3 and N % 2 == 0)
                    if on_act:
                        nc.scalar.activation(dst, cps_l[j][:], Copy)
                    else:
                        nc.vector.tensor_copy(dst, cps_l[j][:])
                # S6: output writes. Edge pairs go out per-subtile (the first
                # writes start a subtile earlier; the final write is half as
                # long); steady-state pairs share one 1 MiB write.
                pair = N // 2
                edge = pair <= 2 or pair >= NSUB // 2 - 2
                r0 = pair * 256
                if edge:
                    nc.sync.dma_start(
                        out[r0 + half * 128:r0 + (half + 1) * 128, :],
                        o_sb[:, half, :])
                elif half == 1:
                    nc.sync.dma_start(
                        out[r0:r0 + 256, :].rearrange(
                            "(s p) o -> p s o", p=128),
                        o_sb[:])

    nc.compile()
    return nc


def _shard_xT(x, c):
    return np.ascontiguousarray(x[c * TPC:(c + 1) * TPC].T).astype(F8)


_runner = None


def _get_runner(nc):
    """Build the sharded PJRT callable once; reuse across kernel() calls.

    Mirrors bass2jax.run_bass_via_pjrt's multi-core branch, but caches the
    jitted function so repeat calls skip retrace/recompile. Falls back to
    the stock path (handled by caller) on any failure.
    """
    global _runner
    if _runner is not None:
        return _runner
    import jax
    from jax.experimental.shard_map import shard_map
    from jax.sharding import Mesh, PartitionSpec

    from concourse import bass2jax, mybir as _mb

    bass2jax.install_neuronx_cc_hook()
    partition_name = (nc.partition_id_tensor.name
                      if nc.partition_id_tensor else None)
    in_names, out_names, out_avals = [], [], []
    for alloc in nc.m.functions[0].allocations:
        if not isinstance(alloc, _mb.MemoryLocationSet):
            continue
        name = alloc.memorylocations[0].name
        if alloc.kind == "ExternalInput":
            if name != partition_name:
                in_names.append(name)
        elif alloc.kind == "ExternalOutput":
            out_names.append(name)
            out_avals.append(jax.core.ShapedArray(
                tuple(alloc.tensor_shape), _mb.dt.np(alloc.dtype)))
    n_params = len(in_names)
    n_outs = len(out_avals)
    all_in_names = list(in_names) + list(out_names)
    if partition_name is not None:
        all_in_names.append(partition_name)

    def _body(*args):
        operands = list(args)
        if partition_name is not None:
            operands.append(bass2jax.partition_id_tensor())
        outs = bass2jax._bass_exec_p.bind(
            *operands,
            out_avals=tuple(out_avals),
            in_names=tuple(all_in_names),
            out_names=tuple(out_names),
            lowering_input_output_aliases=(),
            sim_require_finite=True,
            sim_require_nnan=True,
            nc=nc,
        )
        return tuple(outs)

    devices = jax.devices()[:NCORES]
    mesh = Mesh(np.asarray(devices), ("core",))
    specs = (PartitionSpec("core"),) * (n_params + n_outs)
    sharded = jax.jit(
        shard_map(_body, mesh=mesh, in_specs=specs,
                  out_specs=(PartitionSpec("core"),) * n_outs,
                  check_rep=False),
        donate_argnums=tuple(range(n_params, n_params + n_outs)),
        keep_unused=True,
    )
    _runner = (sharded, in_names, out_names, out_avals)
    return _runner


def _run_cached(nc, in_maps):
    sharded, in_names, out_names, out_avals = _get_runner(nc)
    concat_in = [
        np.concatenate([np.asarray(m[name]) for m in in_maps], axis=0)
        for name in in_names
    ]
    concat_zeros = [
        np.zeros((NCORES * a.shape[0], *a.shape[1:]), a.dtype)
        for a in out_avals
    ]
    out_arrs = sharded(*concat_in, *concat_zeros)
    return [
        {name: np.asarray(out_arrs[i]).reshape(NCORES, *out_avals[i].shape)[c]
         for i, name in enumerate(out_names)}
        for c in range(NCORES)
    ]


def kernel(x, W_route, A, Bw, lora_ind):
    global _nc_cache
    x = np.asarray(x, dtype=np.float32).reshape(NTOK, D)
    W_route = np.asarray(W_route, dtype=np.float32)
    A = np.asarray(A, dtype=np.float32)
    Bw = np.asarray(Bw, dtype=np.float32)
    lora_ind = np.asarray(lora_ind).astype(np.int64)

    # [D, 136] fp16: cols 0..127 are A rows in (g, e, r) order, 128.. W_route;
    # re-packed to [128, KD, FE] so each partition's DMA line is contiguous.
    A_all = A.transpose(1, 0, 2, 3).reshape(F, D)
    AWT = np.concatenate([A_all.T, W_route.T], axis=1).astype(np.float16)
    AWT_dev = np.ascontiguousarray(
        AWT.reshape(KD, 128, FE).transpose(1, 0, 2))
    # block-diagonal B^T with SCALING folded in: rows (g,e,r), cols (g,o)
    BTbd = (Bw.transpose(1, 0, 3, 2).reshape(G, E * R, OD)
            * SCALING).astype(np.float16)

    if _nc_cache is None:
        _nc_cache = _build()
    nc = _nc_cache

    with ThreadPoolExecutor(NCORES) as ex:
        xTs = list(ex.map(lambda c: _shard_xT(x, c), range(NCORES)))
    in_maps = [{"xT": xTs[c], "AWT": AWT_dev, "BT": BTbd}
               for c in range(NCORES)]

    try:
        results = _run_cached(nc, in_maps)
    except Exception:  # noqa: BLE001  (fall back to the stock SPMD path)
        global _runner
        _runner = None
        res = run_bass_kernel_spmd(nc, in_maps, core_ids=list(range(NCORES)),
                                   **_RUN_KWARGS)
        results = res.results
    _LAST["results"] = results

    compact = np.concatenate(
        [results[c]["out"] for c in range(NCORES)], axis=0)
    outp = np.zeros((NTOK, OUT), dtype=np.float32)
    outp[:, lora_ind] = compact.astype(np.float32)
    return outp.reshape(B, S, OUT)
